# revision 2
# baseline (speedup 1.0000x reference)
"""Trainium2 Bass kernel for nn_Baseline_9904194584728 (v2, transfer-optimized).

Pipeline: embedding gathers + MLP (293->64->64->64->9) + pnerf scan.

v2 changes vs baseline (2.08 s/iter -> target ~0.4 s/iter). The metric is
dominated by the axon tunnel (~100 ms dispatch floor + ~14 ms/MB shipped,
replicated tensors ship 8x) and by re-tracing a fresh jax.jit every call.
  * The jitted shard_map callable is built ONCE and cached; per call we
    only upload inputs, execute, and fetch the output.
  * The W0-folded embedding tables (KW = kmer_embed @ W0[16:272] + hi/lo
    bf16 packing, SW likewise) are computed on HOST; the 87 MB replicated
    `ket` upload and the on-device table-build phase are gone. Each core
    uploads only its 1/8 chunk of the packed table (340 KB); the kernel
    AllGathers the full 10648x128 table over NeuronLink.
  * pssm ships as int16 fixed point q = round((p-0.5)*2^16) (11 MB instead
    of 22 MB); the device converts to f32 integers and the 2^-16 scale is
    folded into W0[272:293] host-side, the 0.5 shift into the SW table.
    (bf16 pssm is NOT usable: pnerf amplifies h0 noise ~1600x; int16 fixed
    point keeps the end-to-end rel err ~3e-3 vs the 2e-2 gate.)
  * All per-core inputs are packed into ONE [118, 8192] int16 blob (one
    jit parameter, ~1.9 MB/core) using AP.bitcast views on device.
  * The output ships as int16 positions scaled by 2^15 (4.7 MB, error
    ~3e-5 abs) and the zero "donation" buffers for the bass_exec operands
    live device-resident across calls instead of being uploaded.
  * Data-parallel over B across the 8 cores (B_s = 32 per core).
"""

import sys
sys.path.insert(0, "/opt/trn_rl_repo")

import os
import numpy as np
import ml_dtypes
from contextlib import ExitStack

import jax
from jax.sharding import Mesh, PartitionSpec as P, NamedSharding
from jax.experimental.shard_map import shard_map

import concourse.bass as bass
import concourse.tile as tile
from concourse import bacc, mybir
from concourse.bass2jax import (_bass_exec_p, install_neuronx_cc_hook,
                                partition_id_tensor)

F32 = mybir.dt.float32
BF16 = mybir.dt.bfloat16
I16 = mybir.dt.int16
AL = mybir.AluOpType
AF = mybir.ActivationFunctionType

NCORE = 8
L = 1024
B = 256
BS = B // NCORE            # 32 batch per core
TOK = L * BS               # 32768 tokens per core
NT = TOK // 512            # 64 batch-tiles of 512
NSUP = 8                   # supertiles of 4096 tokens (gather granularity)
NKMER = 10648
KCH = NKMER // NCORE       # 1331 table rows per core
N3 = 3 * L                 # 3072 chain length
S = 24                     # chunk size (level-1)
C = N3 // S                # 128 chunks
EPS2 = 1e-24
PSCALE = 65536.0           # pssm fixed-point scale (2^16, shifted by 0.5)
OSCALE = 32768.0           # output fixed-point scale (2^15)
KPH2 = os.environ.get("KPH2", "CGBS")

# blob row map ([118, 8192] int16 per core)
R_PSSM = 0      # 84 rows: row 21q+f  <->  pssm pack row 32q+f
R_KIDX = 84     # 4 rows  = [16, 2048] wrapped kmer indices
R_SIDX = 88     # 4 rows  = [16, 2048] wrapped seq indices
R_SWP = 92      # cols 0:2560  = [20, 128] bf16 SW hi|lo table
R_IDK = 93      # 1 row  = [128, 64] bf16 stacked identity
R_WE = 94       # 1 row  = [64, 64] f32 We
R_MISC = 95     # cols 0:1152 w1 [64,9] f32; 1152:1280 be [64,1];
                # 1280:1298 b1 [9,1]; 1298:1322 id12 [1,12]
R_W0P = 96      # cols 0:2688  = [21, 64] f32 W0[272:293] * 2^-16
R_KWP = 97      # 21 rows = [1331, 128] bf16 KW hi|lo chunk (flat + pad)
RB = 118


# --------------------------------------------------------------------------
# device kernel builder
# --------------------------------------------------------------------------

def _compose_views(t_ap, mode):
    """Return (pcol, arow, outv, col3) view factories for a [128, 384]
    transform tile.

    mode 'mj':  free = m*32 + lane   (m-major; lane = j or ch, 32 lanes)
    mode 'lm':  free = lane*12 + m   (lane-major)
    All views have dims (b, a, lane) with counts (4, 3, 32).
    """
    if mode == 'mj':
        def pcol(cc):
            v = t_ap[:, 3 * cc * 32:(3 * cc + 3) * 32]
            v = v.rearrange("p (a j) -> p a j", a=3)
            return v.unsqueeze(1).broadcast_to([128, 4, 3, 32])

        def arow(cc):
            v = t_ap[:, 0:384].rearrange("p (b three j) -> p b three j",
                                         b=4, three=3)
            v = v[:, :, cc, :]
            return v.unsqueeze(2).broadcast_to([128, 4, 3, 32])

        def outv():
            return t_ap[:, 0:384].rearrange("p (b a j) -> p b a j", b=4, a=3)

        def col3():
            return t_ap[:, 288:384]
    else:  # 'lm'
        def pcol(cc):
            v = t_ap[:, 0:384].rearrange("p (lan m) -> p lan m", lan=32)
            v = v[:, :, 3 * cc:3 * cc + 3]          # [p, lan, a]
            v = v.transpose([0, 2, 1])              # [p, a, lan]
            return v.unsqueeze(1).broadcast_to([128, 4, 3, 32])

        def arow(cc):
            v = t_ap[:, 0:384].rearrange("p (lan b three) -> p lan b three",
                                         lan=32, b=4)
            v = v[:, :, :, cc]                      # [p, lan, b]
            v = v.transpose([0, 2, 1])              # [p, b, lan]
            return v.unsqueeze(2).broadcast_to([128, 4, 3, 32])

        def outv():
            v = t_ap[:, 0:384].rearrange("p (lan b a) -> p lan b a",
                                         lan=32, b=4)
            return v.transpose([0, 2, 3, 1])        # [p, b, a, lan]

        def col3():
            v = t_ap[:, 0:384].rearrange("p (lan m) -> p lan m", lan=32)
            return v[:, :, 9:12]                    # [p, lan, a]
    return pcol, arow, outv, col3


def _emit_compose(nc, dst, Pt, A, tmpM, tmp2, mode):
    """dst = Pt o A for transform tiles [128, 384] in the given layout."""
    Pp, _, _, Pc3 = _compose_views(Pt, mode)
    _, Aa, _, _ = _compose_views(A, mode)
    _, _, Mo, _ = _compose_views(tmpM, mode)
    _, _, To, _ = _compose_views(tmp2, mode)
    Dp, _, Do, Dc3 = _compose_views(dst, mode)
    nc.vector.tensor_tensor(Mo(), Pp(0), Aa(0), AL.mult)
    nc.vector.tensor_tensor(To(), Pp(1), Aa(1), AL.mult)
    nc.vector.tensor_tensor(tmpM[:, 0:384], tmpM[:, 0:384], tmp2[:, 0:384],
                            AL.add)
    nc.vector.tensor_tensor(To(), Pp(2), Aa(2), AL.mult)
    nc.vector.tensor_tensor(dst[:, 0:384], tmpM[:, 0:384], tmp2[:, 0:384],
                            AL.add)
    nc.vector.tensor_tensor(Dc3(), Dc3(), Pc3(), AL.add)


def build_nc():
    nc = bacc.Bacc("TRN2", target_bir_lowering=False, debug=False,
                   num_devices=NCORE)

    # ---------------- I/O ----------------
    d_blob = nc.declare_dram_parameter("blob", [RB, 8192], I16, isOutput=False)
    o_scan = nc.declare_dram_parameter("o_scan", [128, 2304], I16,
                                       isOutput=True)

    # ---------------- internal DRAM ----------------
    kwp_bnc = nc.dram_tensor("kwp_bnc", [KCH, 128], BF16)
    kwp = nc.dram_tensor("kwp_full", [NKMER, 128], BF16)
    swp = nc.dram_tensor("swp_d", [20, 128], BF16)
    srf_d = nc.dram_tensor("srf_d", [9, TOK], F32)
    d_tc2 = nc.dram_tensor("d_tc2", [128, 384], F32)
    d_g = nc.dram_tensor("d_g", [128, 12], F32)
    d_b2 = nc.dram_tensor("d_b2", [128, 384], F32)

    blob = d_blob  # AP helper

    with ExitStack() as ctx:
        tc = ctx.enter_context(tile.TileContext(nc))

        # ---------------- phase A': unpack blob + table allgather --------
        # kwp chunk -> bounce -> AllGather (start this first; gathers wait
        # on it while the rest of the setup proceeds)
        kwp_src = blob[R_KWP:RB, :] \
            .rearrange("r (k c) -> (r k) c", c=128)[0:KCH, :].bitcast(BF16)
        nc.sync.dma_start(kwp_bnc[:, :], kwp_src)
        if "C" in KPH2:
            nc.gpsimd.collective_compute(
                "AllGather", AL.bypass,
                replica_groups=[list(range(NCORE))],
                ins=[kwp_bnc.ap().opt()], outs=[kwp.ap().opt()])
        # SW table DRAM->DRAM
        swp_src = blob[R_SWP:R_SWP + 1, 0:2560] \
            .rearrange("o (t c) -> (o t) c", c=128).bitcast(BF16)
        nc.sync.dma_start(swp[:, :], swp_src)

        # persistent pool
        pw = ctx.enter_context(tc.tile_pool(name="pw", bufs=1))
        t_w0p4 = pw.tile([128, 64], F32, tag="w0p4")
        t_we = pw.tile([64, 64], F32, tag="we")
        t_w1 = pw.tile([64, 9], F32, tag="w1")
        t_be = pw.tile([64, 1], F32, tag="be")
        t_b1 = pw.tile([9, 1], F32, tag="b1")
        t_idk = pw.tile([128, 64], BF16, tag="idk")
        t_kidx = pw.tile([128, TOK // 16], I16, tag="kidx")
        t_sidx = pw.tile([128, TOK // 16], I16, tag="sidx")
        t_pssm = pw.tile([128, 8192], F32, tag="pssm")

        # indices: [16, 2048] -> replicate to 128 partitions (plain DMAs;
        # split-partition dst patterns scramble/crash on HW)
        kidx_src = blob[R_KIDX:R_KIDX + 4, :] \
            .rearrange("r (s c) -> (r s) c", s=4)
        sidx_src = blob[R_SIDX:R_SIDX + 4, :] \
            .rearrange("r (s c) -> (r s) c", s=4)
        for g in range(8):
            nc.sync.dma_start(t_kidx[16 * g:16 * g + 16, :], kidx_src)
            nc.sync.dma_start(t_sidx[16 * g:16 * g + 16, :], sidx_src)

        # small weights via f32 bitcast views
        nc.sync.dma_start(
            t_we[:], blob[R_WE:R_WE + 1, :].bitcast(F32)
            .rearrange("o (p c) -> (o p) c", c=64))
        nc.sync.dma_start(
            t_w1[:], blob[R_MISC:R_MISC + 1, 0:1152].bitcast(F32)
            .rearrange("o (p c) -> (o p) c", c=9))
        nc.sync.dma_start(
            t_be[:], blob[R_MISC:R_MISC + 1, 1152:1280].bitcast(F32)
            .rearrange("o (p c) -> (o p) c", c=1))
        nc.sync.dma_start(
            t_b1[:], blob[R_MISC:R_MISC + 1, 1280:1298].bitcast(F32)
            .rearrange("o (p c) -> (o p) c", c=1))
        nc.sync.dma_start(
            t_idk[:], blob[R_IDK:R_IDK + 1, :].bitcast(BF16)
            .rearrange("o (p c) -> (o p) c", c=64))
        w0p_src = blob[R_W0P:R_W0P + 1, 0:2688].bitcast(F32) \
            .rearrange("o (f c) -> (o f) c", c=64)
        for q in range(4):
            nc.sync.dma_start(t_w0p4[32 * q:32 * q + 21, :], w0p_src)

        # pssm int16 -> f32 (integer-valued; scale folded into w0p4)
        with ExitStack() as pctx:
            pp = pctx.enter_context(tc.tile_pool(name="pp", bufs=1))
            st = pp.tile([128, 8192], I16, tag="st")
            for q in range(4):
                nc.sync.dma_start(st[32 * q:32 * q + 21, :],
                                  blob[R_PSSM + 21 * q:R_PSSM + 21 * q + 21, :])
            for q in range(4):
                nc.vector.tensor_copy(t_pssm[32 * q:32 * q + 21, :],
                                      st[32 * q:32 * q + 21, :])

        # ---------------- phase B: MLP ----------------
        with ExitStack() as bctx:
            gp = bctx.enter_context(tc.tile_pool(name="gp", bufs=2))
            hb = bctx.enter_context(tc.tile_pool(name="hb", bufs=3))
            bps = bctx.enter_context(
                tc.tile_pool(name="bps", bufs=3, space="PSUM"))
            sps = bctx.enter_context(
                tc.tile_pool(name="sps", bufs=2, space="PSUM"))
            sf = bctx.enter_context(tc.tile_pool(name="sf", bufs=2))

            GW = TOK // NSUP                     # 4096 idx per gather
            for sup in range(NSUP if ("G" in KPH2) else 0):
                kg = gp.tile([128, GW], BF16, tag="kg")
                sg = gp.tile([128, GW], BF16, tag="sg")
                isl = slice(sup * (GW // 16), (sup + 1) * (GW // 16))
                nc.gpsimd.dma_gather(
                    kg[:].rearrange("p (one n) -> p one n", one=1),
                    kwp[:, :], t_kidx[:, isl], num_idxs=GW, num_idxs_reg=GW,
                    elem_size=128, transpose=True, single_packet=False)
                nc.gpsimd.dma_gather(
                    sg[:].rearrange("p (one n) -> p one n", one=1),
                    swp[:, :], t_sidx[:, isl], num_idxs=GW, num_idxs_reg=GW,
                    elem_size=128, transpose=True, single_packet=False)
                srfS = sf.tile([9, GW], F32, tag="srfS")
                if "B" not in KPH2:
                    nc.vector.memset(srfS[:], 0.0)
                if "D" in KPH2 and sup == 0:
                    dbg = pw.tile([128, 2304], I16, tag="dbg")
                    pK = bps.tile([64, 512], F32, tag="ph")
                    nc.tensor.matmul(pK[:], t_idk[:], kg[:, 0:512],
                                     start=True, stop=True)
                    nc.vector.tensor_scalar_mul(
                        dbg[0:64, 512:1024], pK[:], 4096.0)
                    pS = bps.tile([64, 512], F32, tag="ph")
                    nc.tensor.matmul(pS[:], t_idk[:], sg[:, 0:512],
                                     start=True, stop=True)
                    nc.vector.tensor_scalar_mul(
                        dbg[0:64, 1024:1536], pS[:], 4096.0)
                    pP = bps.tile([64, 512], F32, tag="ph")
                    nc.tensor.matmul(pP[:], t_w0p4[0:21, :],
                                     t_pssm[0:21, 0:512], start=True,
                                     stop=True)
                    nc.vector.tensor_scalar_mul(
                        dbg[0:64, 1536:2048], pP[:], 4096.0)
                for tp in range((NT // NSUP) if ("B" in KPH2) else 0):     # 8 batch-tiles per supertile
                    t = sup * (NT // NSUP) + tp
                    q, r = t % 4, t // 4
                    csl = slice(tp * 512, (tp + 1) * 512)
                    ph0 = bps.tile([64, 512], F32, tag="ph")
                    nc.tensor.matmul(ph0[:], t_idk[:], kg[:, csl],
                                     start=True, stop=False)
                    nc.tensor.matmul(ph0[:], t_idk[:], sg[:, csl],
                                     start=False, stop=False)
                    nc.tensor.matmul(
                        ph0[:], t_w0p4[32 * q:32 * q + 21, :],
                        t_pssm[32 * q:32 * q + 21, 512 * r:512 * r + 512],
                        start=False, stop=True,
                        tile_position=(32 * q, 0))
                    h0 = hb.tile([64, 512], F32, tag="h0")
                    nc.scalar.activation(h0[:], ph0[:], AF.Copy)
                    if "D" in KPH2 and t == 0:
                        nc.vector.tensor_scalar_mul(
                            dbg[0:64, 0:512], h0[:], 4096.0)
                    ph1 = bps.tile([64, 512], F32, tag="ph")
                    nc.tensor.matmul(ph1[:], t_we[:], h0[:], start=True,
                                     stop=True)
                    h1 = hb.tile([64, 512], F32, tag="h1")
                    nc.vector.tensor_scalar(h1[:], ph1[:], t_be[:], 0.0,
                                            AL.add, AL.max)
                    ph2 = bps.tile([64, 512], F32, tag="ph")
                    nc.tensor.matmul(ph2[:], t_we[:], h1[:], start=True,
                                     stop=True)
                    h2 = hb.tile([64, 512], F32, tag="h2")
                    nc.scalar.activation(h2[:], ph2[:], AF.Relu, bias=t_be[:],
                                         scale=1.0)
                    ps3 = sps.tile([9, 512], F32, tag="ps3")
                    nc.tensor.matmul(ps3[:], t_w1[:], h2[:], start=True,
                                     stop=True)
                    nc.vector.tensor_scalar(srfS[:, csl], ps3[:], t_b1[:],
                                            None, AL.add)
                nc.sync.dma_start(srf_d[:, sup * GW:(sup + 1) * GW], srfS[:])

        # ---------------- phase C: scan ----------------
        cp = ctx.enter_context(tc.tile_pool(name="cp", bufs=1))
        ct_all = cp.tile([128, 2304], F32, tag="ct")
        A_all = cp.tile([128, 24 * 384], F32, tag="Aall")
        q_all = cp.tile([128, 2304], F32, tag="qall")
        p_all = cp.tile([128, 2304], F32, tag="pall")
        sq_all = cp.tile([128, 2304], F32, tag="sqall")
        tmp768a = cp.tile([128, 768], F32, tag="t768a")
        tmp768b = cp.tile([128, 768], F32, tag="t768b")
        n2t = cp.tile([128, 768], F32, tag="n2")
        n2ct = cp.tile([128, 768], F32, tag="n2c")
        rnt = cp.tile([128, 768], F32, tag="rn")
        rnct = cp.tile([128, 768], F32, tag="rnc")
        t_idtf = cp.tile([128, 384], F32, tag="idtf")
        nc.sync.dma_start(
            t_idtf[:].rearrange("p (l m) -> p l m", l=32),
            blob[R_MISC:R_MISC + 1, 1298:1322].bitcast(F32)
            .unsqueeze(0).broadcast_to([128, 32, 12]))

        if "S" not in KPH2:
            zf = cp.tile([128, 2304], I16, tag="zf")
            nc.vector.memset(zf[:], 0)
            nc.sync.dma_start(o_scan[:, :], zf[:])
            nc.compile()
            return nc
        # C0: permute srf -> ct_all [c, (k*3+x)*32 + j]
        srf_r = srf_d.ap().rearrange("(r x) (c k1 j) -> r x c k1 j",
                                     r=3, x=3, c=128, k1=8)
        ct_r = ct_all[:].rearrange("p (k1 k2 x j) -> p k1 k2 x j",
                                   k1=8, k2=3, x=3)
        for k2 in range(3):
            for x in range(3):
                src = srf_r[k2, x]                       # [c, k1, j]
                nc.sync.dma_start(ct_r[:, :, k2, x, :], src)

        # C1: pointwise transform build
        ctv4 = ct_all[:].rearrange("p (k x j) -> p k x j", k=24, x=3)
        sqv4 = sq_all[:].rearrange("p (k x j) -> p k j x", k=24, x=3)
        Af = A_all[:].rearrange("p (k m j) -> p k m j", k=24, m=12)
        n2v = n2t[:].rearrange("p (k j) -> p k j", k=24)
        n2cv = n2ct[:].rearrange("p (k j) -> p k j", k=24)
        rnv3 = rnt[:].rearrange("p (k j) -> p k j", k=24).unsqueeze(2) \
                     .broadcast_to([128, 24, 3, 32])
        rncv = rnct[:].rearrange("p (k j) -> p k j", k=24)

        def ctx_(x):
            return ctv4[:, :, x, :]

        nc.scalar.activation(sq_all[:], ct_all[:], AF.Square)
        nc.vector.tensor_reduce(n2v.unsqueeze(-1), sqv4, mybir.AxisListType.X,
                                AL.add)
        nc.vector.tensor_reduce(n2cv.unsqueeze(-1), sqv4[:, :, :, 1:3],
                                mybir.AxisListType.X, AL.add)
        nc.vector.tensor_scalar_max(n2t[:], n2t[:], EPS2)
        nc.vector.tensor_scalar_max(n2ct[:], n2ct[:], EPS2)
        nc.scalar.activation(tmp768a[:], n2t[:], AF.Sqrt)
        nc.scalar.activation(tmp768b[:], n2ct[:], AF.Sqrt)
        nc.vector.reciprocal_approx_accurate(rnt[:], tmp768a[:],
                                             sq_all[:, 0:768])
        nc.vector.reciprocal_approx_accurate(rnct[:], tmp768b[:],
                                             sq_all[:, 768:1536])
        nc.vector.tensor_tensor(Af[:, :, 0:3, :], ctv4, rnv3, AL.mult)
        nc.scalar.activation(Af[:, :, 9:12, :], ctv4, AF.Copy)
        nc.vector.tensor_scalar_mul(Af[:, :, 6, :], ctx_(0), 0.0)
        nc.vector.scalar_tensor_tensor(Af[:, :, 7, :], ctx_(2), -1.0, rncv,
                                       AL.mult, AL.mult)
        nc.vector.tensor_tensor(Af[:, :, 8, :], ctx_(1), rncv, AL.mult)
        nc.vector.tensor_tensor(Af[:, :, 3, :], Af[:, :, 7, :],
                                Af[:, :, 2, :], AL.mult)
        nc.vector.tensor_tensor(tmp768a[:].rearrange("p (k j) -> p k j", k=24),
                                Af[:, :, 8, :], Af[:, :, 1, :], AL.mult)
        nc.vector.tensor_tensor(Af[:, :, 3, :], Af[:, :, 3, :],
                                tmp768a[:].rearrange("p (k j) -> p k j", k=24),
                                AL.subtract)
        nc.vector.tensor_tensor(Af[:, :, 4, :], Af[:, :, 8, :],
                                Af[:, :, 0, :], AL.mult)
        nc.vector.scalar_tensor_tensor(Af[:, :, 5, :], Af[:, :, 7, :], -1.0,
                                       Af[:, :, 0, :], AL.mult, AL.mult)

        # C2: level-1 scan (23 steps over k)
        Pa = cp.tile([128, 384], F32, tag="Pa")
        Pb = cp.tile([128, 384], F32, tag="Pb")
        tmpM = cp.tile([128, 384], F32, tag="tmpM")
        tmp2 = cp.tile([128, 384], F32, tag="tmp2")
        nc.scalar.activation(Pa[:], A_all[:, 0:384], AF.Copy)
        nc.scalar.activation(q_all[:, 0:96], A_all[:, 288:384], AF.Copy)
        cur, nxt = Pa, Pb
        for k in range(1, S):
            Ak = A_all[:, k * 384:(k + 1) * 384]
            _emit_compose(nc, nxt, cur, Ak, tmpM, tmp2, 'mj')
            nc.scalar.activation(q_all[:, k * 96:(k + 1) * 96],
                                 nxt[:, 288:384], AF.Copy)
            cur, nxt = nxt, cur
        Pfin = cur

        # C3: level-2 (chunk-carry exclusive prefix)
        Palt = cp.tile([128, 384], F32, tag="Palt")
        nc.vector.tensor_copy(
            Palt[:].rearrange("p (j m) -> p j m", j=32),
            Pfin[:].rearrange("p (m j) -> p m j", m=12).transpose([0, 2, 1]))
        nc.sync.dma_start(d_tc2[:, :], Palt[:])
        T2 = cp.tile([128, 384], F32, tag="T2")
        tc2r = d_tc2.ap().rearrange("c (j m) -> c j m", j=32)
        for cl in range(4):
            src = tc2r[32 * cl:32 * cl + 32].transpose([1, 0, 2])  # [j, ch, m]
            nc.sync.dma_start(
                T2[32 * cl:32 * cl + 32, :]
                .rearrange("p (ch m) -> p ch m", ch=32), src)

        chS = cp.tile([128, 384], F32, tag="chS")
        nc.vector.tensor_copy(chS[:], T2[:])

        def lane_views(t_ap, lanes):
            lo, n, step = lanes
            base = t_ap[:, 0:384].rearrange("p (lan m) -> p lan m", lan=32)
            idx = base[:, lo:lo + (n - 1) * step + 1:step, :] if step > 1 \
                else base[:, lo:lo + n, :]
            return idx  # [p, n, 12]

        def compose_lanes(dst_l, P_l, A_l, nl):
            def mk(v):
                pc = v[:, :, 0:9].rearrange("p n (c a) -> p n c a", c=3)

                def pcol(cc):
                    return pc[:, :, cc, :].transpose([0, 2, 1]) \
                        .unsqueeze(1).broadcast_to([128, 4, 3, nl])

                ar = v.rearrange("p n (b three) -> p n b three", b=4)

                def arow(cc):
                    return ar[:, :, :, cc].transpose([0, 2, 1]) \
                        .unsqueeze(2).broadcast_to([128, 4, 3, nl])

                def outv():
                    return v.rearrange("p n (b a) -> p b a n", b=4)

                def col3():
                    return v[:, :, 9:12]
                return pcol, arow, outv, col3

            Pp, _, _, Pc3 = mk(P_l)
            _, Aa, _, _ = mk(A_l)
            tM = lane_views(tmpM, (0, nl, 1))
            t2 = lane_views(tmp2, (0, nl, 1))
            _, _, Mo, _ = mk(tM)
            _, _, To, _ = mk(t2)
            _, _, Do, Dc3 = mk(dst_l)
            nc.vector.tensor_tensor(Mo(), Pp(0), Aa(0), AL.mult)
            nc.vector.tensor_tensor(To(), Pp(1), Aa(1), AL.mult)
            nc.vector.tensor_tensor(Mo(), Mo(), To(), AL.add)
            nc.vector.tensor_tensor(To(), Pp(2), Aa(2), AL.mult)
            nc.vector.tensor_tensor(Do(), Mo(), To(), AL.add)
            nc.vector.tensor_tensor(Dc3(), Dc3(), Pc3(), AL.add)

        for w in range(1, 8):
            prev = lane_views(chS, (w - 1, 4, 8))
            curA = lane_views(T2, (w, 4, 8))
            dst = lane_views(chS, (w, 4, 8))
            compose_lanes(dst, prev, curA, 4)

        btot = cp.tile([128, 48], F32, tag="btot")
        btv = btot[:].rearrange("p (n m) -> p n m", n=4)
        nc.vector.tensor_copy(btv[:, 0:1, :], lane_views(chS, (7, 1, 1)))
        for blk in range(1, 4):
            compose_lanes(btv[:, blk:blk + 1, :], btv[:, blk - 1:blk, :],
                          lane_views(chS, (blk * 8 + 7, 1, 1)), 1)

        Pchi = cp.tile([128, 384], F32, tag="Pchi")
        nc.vector.tensor_copy(Pchi[:, 0:96], chS[:, 0:96])
        for blk in range(1, 4):
            bview = btv[:, blk - 1:blk, :].broadcast_to([128, 8, 12])
            compose_lanes(lane_views(Pchi, (blk * 8, 8, 1)), bview,
                          lane_views(chS, (blk * 8, 8, 1)), 8)

        Pche = cp.tile([128, 384], F32, tag="Pche")
        nc.vector.tensor_copy(Pche[:, 0:12], t_idtf[:, 0:12])
        nc.vector.tensor_copy(Pche[:, 12:384], Pchi[:, 0:372])

        nc.sync.dma_start(d_g[:, :], Pchi[:, 372:384])
        G4 = cp.tile([128, 48], F32, tag="G4")
        for clp in range(4):
            src = d_g.ap()[32 * clp:32 * clp + 32, :]
            src = src.unsqueeze(0).broadcast_to([4, 32, 12])
            nc.sync.dma_start(G4[:, clp * 12:(clp + 1) * 12], src)
        g4v = G4[:].rearrange("p (n m) -> p n m", n=4)
        P01t = cp.tile([128, 12], F32, tag="P01t")
        P012t = cp.tile([128, 12], F32, tag="P012t")
        compose_lanes(P01t[:].unsqueeze(1), g4v[:, 0:1, :], g4v[:, 1:2, :], 1)
        compose_lanes(P012t[:].unsqueeze(1), P01t[:].unsqueeze(1),
                      g4v[:, 2:3, :], 1)
        Pexcl = cp.tile([128, 12], F32, tag="Pexcl")
        nc.vector.tensor_copy(Pexcl[0:32, :], t_idtf[0:32, 0:12])
        nc.vector.tensor_copy(Pexcl[32:64, :], G4[32:64, 0:12])
        nc.vector.tensor_copy(Pexcl[64:96, :], P01t[64:96, :])
        nc.vector.tensor_copy(Pexcl[96:128, :], P012t[96:128, :])

        Bcj = cp.tile([128, 384], F32, tag="Bcj")
        compose_lanes(lane_views(Bcj, (0, 32, 1)),
                      Pexcl[:].unsqueeze(1).broadcast_to([128, 32, 12]),
                      lane_views(Pche, (0, 32, 1)), 32)
        nc.sync.dma_start(d_b2[:, :], Bcj[:])
        Bch = cp.tile([128, 384], F32, tag="Bch")
        b2r = d_b2.ap().rearrange("p (ch m) -> p ch m", ch=32)
        for cl in range(4):
            src = b2r[32 * cl:32 * cl + 32].transpose([1, 0, 2])  # [ch, j, m]
            nc.sync.dma_start(
                Bch[32 * cl:32 * cl + 32, :]
                .rearrange("p (j m) -> p j m", j=32), src)

        # C4: apply  p = B.t + B.R @ q ; then scale -> int16 out
        qv = q_all[:].rearrange("p (k x j) -> p k x j", k=24, x=3)
        Bv = Bch[:].rearrange("p (j m) -> p j m", j=32)
        pv = p_all[:].rearrange("p (k a j) -> p k a j", k=24, a=3)
        tA = sq_all[:]  # reuse as scratch [128, 2304]
        tAv = tA.rearrange("p (k a j) -> p k a j", k=24, a=3)
        tB = ct_all[:]  # reuse as scratch
        tBv = tB.rearrange("p (k a j) -> p k a j", k=24, a=3)

        def qx(cc):
            return qv[:, :, cc, :].unsqueeze(2).broadcast_to([128, 24, 3, 32])

        def bcol(cc):
            v = Bv[:, :, 3 * cc:3 * cc + 3].transpose([0, 2, 1])  # [p,a,j]
            return v.unsqueeze(1).broadcast_to([128, 24, 3, 32])

        nc.vector.tensor_tensor(tAv, qx(0), bcol(0), AL.mult)
        nc.vector.tensor_tensor(tBv, qx(1), bcol(1), AL.mult)
        nc.vector.tensor_tensor(tAv, tAv, tBv, AL.add)
        nc.vector.tensor_tensor(tBv, qx(2), bcol(2), AL.mult)
        nc.vector.tensor_tensor(tAv, tAv, tBv, AL.add)
        nc.vector.tensor_tensor(pv, tAv, bcol(3), AL.add)
        if "D" in KPH2:
            nc.sync.dma_start(o_scan[:, :], dbg[:])
        else:
            t_oi16 = cp.tile([128, 2304], I16, tag="oi16")
            nc.vector.tensor_scalar_mul(t_oi16[:], p_all[:], OSCALE)
            nc.sync.dma_start(o_scan[:, :], t_oi16[:])

    nc.compile()
    return nc


# --------------------------------------------------------------------------
# host wrapper: cached jit runner
# --------------------------------------------------------------------------

_RT = {}


def _get_rt():
    if _RT:
        return _RT
    install_neuronx_cc_hook()
    nc = build_nc()

    in_names = []
    out_names = []
    out_avals = []
    pid_name = nc.partition_id_tensor.name if nc.partition_id_tensor else None
    for alloc in nc.m.functions[0].allocations:
        if not isinstance(alloc, mybir.MemoryLocationSet):
            continue
        name = alloc.memorylocations[0].name
        if alloc.kind == "ExternalInput":
            if name != pid_name:
                in_names.append(name)
        elif alloc.kind == "ExternalOutput":
            out_names.append(name)
            shape = tuple(alloc.tensor_shape)
            dtype = mybir.dt.np(alloc.dtype)
            out_avals.append(jax.core.ShapedArray(shape, dtype))
    n_params = len(in_names)
    n_outs = len(out_avals)
    in_names_full = list(in_names) + list(out_names)
    if pid_name is not None:
        in_names_full.append(pid_name)

    def _body(*args):
        operands = list(args)
        if pid_name is not None:
            operands.append(partition_id_tensor())
        outs = _bass_exec_p.bind(
            *operands,
            out_avals=tuple(out_avals),
            in_names=tuple(in_names_full),
            out_names=tuple(out_names),
            lowering_input_output_aliases=(),
            sim_require_finite=True,
            sim_require_nnan=True,
            nc=nc)
        return tuple(outs)

    mesh = Mesh(np.asarray(jax.devices()[:NCORE]), ("core",))
    in_specs = (P("core"),) * (n_params + n_outs)
    out_specs = (P("core"),) * n_outs
    fn = jax.jit(shard_map(_body, mesh=mesh, in_specs=in_specs,
                           out_specs=out_specs, check_rep=False),
                 keep_unused=True)
    # the out-buffer operand is never read by the kernel (o_scan is fully
    # written); keep it device-resident so it costs nothing per call
    oz = jax.device_put(
        np.zeros((NCORE * 128, 2304), np.int16),
        NamedSharding(mesh, P("core")))
    _RT.update(nc=nc, fn=fn, oz=oz, mesh=mesh)
    return _RT


def run_device(blob_global):
    """blob_global: [NCORE*RB, 8192] int16 -> [NCORE*128, 2304] int16."""
    rt = _get_rt()
    out = rt["fn"](blob_global, rt["oz"])[0]
    return np.asarray(out)


# --------------------------------------------------------------------------
# host pre/post processing
# --------------------------------------------------------------------------

def _bf16_hilo(x):
    """f32 [n, 64] -> [n, 128] bf16 hi|lo (exact f32 reconstruction)."""
    hi = x.astype(ml_dtypes.bfloat16)
    lo = (x - hi.astype(np.float32)).astype(ml_dtypes.bfloat16)
    return np.concatenate([hi, lo], axis=1)


def _wrap_idx(flat_idx):
    """int array (TOK,) -> [16, TOK/16] int16 wrapped."""
    return flat_idx.astype(np.int16).reshape(TOK // 16, 16).T


def make_blob(inputs):
    seq = np.asarray(inputs["seq"])
    kmer = np.asarray(inputs["kmer"])
    pssm = np.asarray(inputs["pssm"], dtype=np.float32)
    seq_embed = np.asarray(inputs["seq_embed"], dtype=np.float32)
    kmer_embed = np.asarray(inputs["kmer_embed"], dtype=np.float32)
    W0 = np.asarray(inputs["W0"], dtype=np.float32)
    b0 = np.asarray(inputs["b0"], dtype=np.float32)
    We = np.asarray(inputs["We"], dtype=np.float32)
    be = np.asarray(inputs["be"], dtype=np.float32)
    W1 = np.asarray(inputs["W1"], dtype=np.float32)
    b1 = np.asarray(inputs["b1"], dtype=np.float32)

    W0p = W0[272:293]                                       # (21, 64)
    KW = kmer_embed @ W0[16:272]                            # (10648, 64)
    SW = seq_embed @ W0[:16] + b0 + 0.5 * W0p.sum(axis=0)   # (20, 64)
    kwp = _bf16_hilo(KW)                                    # (10648, 128) bf16
    swp = _bf16_hilo(SW)                                    # (20, 128) bf16

    pq = np.clip(np.round((pssm - 0.5) * PSCALE), -32767, 32767) \
        .astype(np.int16)                                   # (L, B, 21)

    # shared blob rows (same content on every core)
    shared = np.zeros((RB - R_SWP, 8192), np.int16)         # rows 92..117
    shared[0, 0:2560] = swp.reshape(-1).view(np.int16)
    identk = np.tile(np.eye(64, dtype=ml_dtypes.bfloat16), (2, 1))
    shared[R_IDK - R_SWP, :] = identk.reshape(-1).view(np.int16)
    shared[R_WE - R_SWP, :] = np.ascontiguousarray(We) \
        .reshape(-1).view(np.int16)
    misc = shared[R_MISC - R_SWP]
    misc[0:1152] = np.ascontiguousarray(W1).reshape(-1).view(np.int16)
    misc[1152:1280] = be.astype(np.float32).view(np.int16)
    misc[1280:1298] = b1.astype(np.float32).view(np.int16)
    id12 = np.array([1, 0, 0, 0, 1, 0, 0, 0, 1, 0, 0, 0], np.float32)
    misc[1298:1322] = id12.view(np.int16)
    shared[R_W0P - R_SWP, 0:2688] = (W0p * (1.0 / PSCALE)) \
        .astype(np.float32).reshape(-1).view(np.int16)

    blob = np.zeros((NCORE * RB, 8192), np.int16)
    for c in range(NCORE):
        bb = blob[c * RB:(c + 1) * RB]
        bsl = slice(c * BS, (c + 1) * BS)
        # pssm rows: 21q+f <-> pack row 32q+f of the baseline layout
        pf = pq[:, bsl, :].reshape(TOK, 21)
        arr = pf.reshape(16, 4, 512, 21)
        for q in range(4):
            bb[21 * q:21 * q + 21] = \
                arr[:, q].transpose(2, 0, 1).reshape(21, 8192)
        bb[R_KIDX:R_KIDX + 4] = \
            _wrap_idx(kmer[:, bsl].reshape(TOK)).reshape(4, 8192)
        bb[R_SIDX:R_SIDX + 4] = \
            _wrap_idx(seq[:, bsl].reshape(TOK)).reshape(4, 8192)
        bb[R_SWP:RB] = shared
        # per-core table chunk overwrites the shared tail rows
        chunk = kwp[c * KCH:(c + 1) * KCH].reshape(-1).view(np.int16)
        kr = bb[R_KWP:RB].reshape(-1)
        kr[0:chunk.size] = chunk
    return blob


def unpack_output(out_global):
    out = np.empty((N3, B, 3), np.float32)
    for c in range(NCORE):
        arr = out_global[c * 128:(c + 1) * 128].astype(np.float32) \
            .reshape(128, 24, 3, 32) * (1.0 / OSCALE)
        out[:, c * BS:(c + 1) * BS, :] = \
            arr.transpose(0, 1, 3, 2).reshape(N3, BS, 3)
    return out


def kernel(**inputs):
    blob = make_blob(inputs)
    out = run_device(blob)
    return unpack_output(out)
